# revision 12
# baseline (speedup 1.0000x reference)
"""Causal self-attention on 8 Trainium2 NeuronCores.

Problem: B=2, S=2048, H=12, D=64, DM=768 (fp32).
Sharding: core = (b, hg): b = core//4 batch, hg = core%4 head-group of 3 heads.
Each core computes q/k/v projections for its 192-feature slice, causal
attention for its 3 heads, and a partial out-projection [2048, 768].
Host sums the 4 partials per batch and adds bo.

Kernel structure (big matmuls in fp32r — 1 cycle/row at N>=256, ~2e-4
relative error vs fp32):
  xT [768, 2048] via PE transposes (fp32 DMA-transpose unsupported)
  qT,kT,vT [*, 2048] = W.T @ x.T  (all projections with N=512 moving dim)
  v_ones [2048, 3*66]: PE-transpose of vT + ones column (denominator trick)
  scores ST [k, q] = kT_h.T @ qT_h -> exp on ACT -> causal mask via
  affine_select on gpsimd (transposed layout: no P transposes needed)
  PV: outT_h[65, q] = v_ones_h.T @ expST (row 64 = softmax denominator),
  emitted per key-chunk right after exp so est tiles recycle fast
  normalization deferred: per block fast-reciprocal of the denominator row,
  gpsimd partition-broadcast, multiply; finally partial = oT.T @ Wo_slice.

HW gotchas encoded here: PSUM pools shared across phases (per-phase pools
serialize at phase boundaries via address WAR); multiple start=True PE
transposes must not share a PSUM bank; partition_broadcast input must
start at partition 0; fp32r compute writes must have even innermost size.
"""

import numpy as np

import concourse.bacc as bacc
import concourse.mybir as mybir
import concourse.tile as tile
from concourse import bass_utils

F32 = mybir.dt.float32
F32R = mybir.dt.float32r
AF = mybir.ActivationFunctionType

B = 2
S = 2048
DM = 768
H = 12
D = 64
SCALE = D ** -0.5

N_CORES = 8
HPC = 3            # heads per core
HS = HPC * D       # 192, feature slice per core
VW = D + 2         # 66, v cols per head: 64 v + ones col + pad (fp32r even-N)
ST_ = S // 128     # 16 seq tiles of 128
KF = DM // 128     # 6 feature chunks of 128
QB = S // 512      # 4 query blocks of 512
KC = S // 128      # 16 key chunks of 128

_PROG = None


def _build_program():
    nc = bacc.Bacc("TRN2", target_bir_lowering=False, debug=False)

    x_d = nc.dram_tensor("x", [S, DM], F32, kind="ExternalInput")
    wq_d = nc.dram_tensor("Wq", [DM, HS], F32, kind="ExternalInput")
    wk_d = nc.dram_tensor("Wk", [DM, HS], F32, kind="ExternalInput")
    wv_d = nc.dram_tensor("Wv", [DM, HS], F32, kind="ExternalInput")
    bq_d = nc.dram_tensor("bq", [HS, 1], F32, kind="ExternalInput")
    bk_d = nc.dram_tensor("bk", [HS, 1], F32, kind="ExternalInput")
    bv_d = nc.dram_tensor("bv", [HS, 1], F32, kind="ExternalInput")
    wo_d = nc.dram_tensor("Wo", [HS, DM], F32, kind="ExternalInput")
    out_d = nc.dram_tensor("out", [S, DM], F32, kind="ExternalOutput")

    def copy_alt(use_scalar, out, in_):
        if use_scalar:
            nc.scalar.copy(out, in_)
        else:
            nc.vector.tensor_copy(out, in_)

    with tile.TileContext(nc) as tc:
        with tc.tile_pool(name="persist", bufs=1) as pp, \
             tc.tile_pool(name="est", bufs=8) as ep, \
             tc.tile_pool(name="psg", bufs=1, space="PSUM") as pg:
            # psg tags: mm512 (proj/scores/outproj psums, 4 banks),
            # pv (2 banks), tr (transposes, 2 banks) = 8 banks total.
            ident = pp.tile([128, 128], F32, tag="ident")
            from concourse.masks import make_identity
            make_identity(nc, ident[:])
            ident_r = pp.tile([128, 128], F32R, tag="ident_r")
            nc.vector.tensor_copy(ident_r[:], ident[:])

            qT01 = pp.tile([128, S], F32R, tag="qT01")   # heads 0,1
            qT2 = pp.tile([64, S], F32R, tag="qT2")      # head 2
            kT01 = pp.tile([128, S], F32R, tag="kT01")
            kT2 = pp.tile([64, S], F32R, tag="kT2")
            # unnormalized attention output (transposed), per head
            oU = [pp.tile([64, S], F32, tag=f"oU{h}", name=f"oU{h}")
                  for h in range(HPC)]
            # normalized f32r version for out-proj
            oT = [pp.tile([64, S], F32R, tag=f"oT{h}", name=f"oT{h}")
                  for h in range(HPC)]
            v_sb = [pp.tile([128, HPC * VW], F32R, tag=f"v{st}", name=f"v{st}")
                    for st in range(ST_)]
            wo_sb = [pp.tile([64, DM], F32R, tag=f"wo{h}", name=f"wo{h}")
                     for h in range(HPC)]
            bq_sb = pp.tile([128, 1], F32, tag="bq01")
            bq2_sb = pp.tile([64, 1], F32, tag="bq2")
            bk_sb = pp.tile([128, 1], F32, tag="bk01")
            bk2_sb = pp.tile([64, 1], F32, tag="bk2")
            bv_sb = pp.tile([128, 1], F32, tag="bv01")
            bv2_sb = pp.tile([64, 1], F32, tag="bv2")

            # small weight loads + casts
            with tc.tile_pool(name="wload", bufs=2) as wl:
                nc.sync.dma_start(bq_sb[:], bq_d.ap()[0:128, :])
                nc.sync.dma_start(bq2_sb[:], bq_d.ap()[128:HS, :])
                nc.sync.dma_start(bk_sb[:], bk_d.ap()[0:128, :])
                nc.sync.dma_start(bk2_sb[:], bk_d.ap()[128:HS, :])
                nc.sync.dma_start(bv_sb[:], bv_d.ap()[0:128, :])
                nc.sync.dma_start(bv2_sb[:], bv_d.ap()[128:HS, :])
                for h in range(HPC):
                    wstg = wl.tile([64, DM], F32, tag="wo_stg")
                    nc.sync.dma_start(wstg[:], wo_d.ap()[h * 64:(h + 1) * 64, :])
                    nc.scalar.copy(wo_sb[h][:], wstg[:])

            wq_sb = [pp.tile([128, HS], F32R, tag=f"wq{kf}", name=f"wq{kf}")
                     for kf in range(KF)]
            wk_sb = [pp.tile([128, HS], F32R, tag=f"wk{kf}", name=f"wk{kf}")
                     for kf in range(KF)]
            wv_sb = [pp.tile([128, HS], F32R, tag=f"wv{kf}", name=f"wv{kf}")
                     for kf in range(KF)]
            with tc.tile_pool(name="wload2", bufs=3) as wl:
                for kf in range(KF):
                    r = slice(kf * 128, (kf + 1) * 128)
                    sq = wl.tile([128, HS], F32, tag="sq")
                    nc.sync.dma_start(sq[:], wq_d.ap()[r, :])
                    nc.scalar.copy(wq_sb[kf][:], sq[:])
                    sk = wl.tile([128, HS], F32, tag="sk")
                    nc.sync.dma_start(sk[:], wk_d.ap()[r, :])
                    nc.scalar.copy(wk_sb[kf][:], sk[:])
                    sv = wl.tile([128, HS], F32, tag="sv")
                    nc.sync.dma_start(sv[:], wv_d.ap()[r, :])
                    nc.scalar.copy(wv_sb[kf][:], sv[:])

            # ones+pad column pair for v_sb (f32r writes must be even width)
            onecol = pp.tile([128, 2], F32, tag="onecol")
            nc.gpsimd.memset(onecol[:], 0.0)
            nc.gpsimd.memset(onecol[:, 0:1], 1.0)

            # ---- Phase A: load x, build xT (+ vT storage) ----
            with tc.tile_pool(name="xT", bufs=1) as xp:
                xT = [xp.tile([128, S], F32R, tag=f"xT{kf}", name=f"xT{kf}")
                      for kf in range(KF)]
                vT01 = xp.tile([128, S], F32R, tag="vT01")
                vT2 = xp.tile([64, S], F32R, tag="vT2")
                with tc.tile_pool(name="xload", bufs=3) as xl:
                    for st in range(ST_):
                        xs = xl.tile([128, DM], F32, tag="xs")
                        nc.sync.dma_start(xs[:], x_d.ap()[st * 128:(st + 1) * 128, :])
                        for kf in range(KF):
                            tp = pg.tile([128, 128], F32, tag="tr", bufs=2)
                            nc.tensor.transpose(
                                tp[:], xs[:, kf * 128:(kf + 1) * 128], ident[:])
                            copy_alt((st * KF + kf) % 2 == 0,
                                     xT[kf][:, st * 128:(st + 1) * 128], tp[:])

                # ---- Phase C: projections qT/kT/vT (N=512 f32r) ----
                for qb in range(QB):
                    cs = slice(qb * 512, (qb + 1) * 512)
                    for (w_sb, b01, b2, t01, t2) in (
                        (wq_sb, bq_sb, bq2_sb, qT01, qT2),
                        (wk_sb, bk_sb, bk2_sb, kT01, kT2),
                        (wv_sb, bv_sb, bv2_sb, vT01, vT2),
                    ):
                        ps = pg.tile([128, 512], F32, tag="mm512", bufs=4)
                        for kf in range(KF):
                            nc.tensor.matmul(
                                ps[:], w_sb[kf][:, 0:128],
                                xT[kf][:, cs],
                                start=(kf == 0), stop=(kf == KF - 1))
                        nc.vector.tensor_scalar_add(t01[:, cs], ps[:], b01[:])
                        ps2 = pg.tile([64, 512], F32, tag="mm512", bufs=4)
                        for kf in range(KF):
                            nc.tensor.matmul(
                                ps2[:], w_sb[kf][:, 128:HS],
                                xT[kf][:, cs],
                                start=(kf == 0), stop=(kf == KF - 1))
                        nc.vector.tensor_scalar_add(t2[:, cs], ps2[:], b2[:])

                    # v_sb tiles for this qb: PE-transpose vT per head.
                    # Each transpose gets its own PSUM slot (start=True
                    # transposes sharing a bank crash the device).
                    for st in range(qb * 4, qb * 4 + 4):
                        sl = slice(st * 128, (st + 1) * 128)
                        vh = [(vT01[0:64, sl], ident_r[0:64, 0:64]),
                              (vT01[64:128, sl], ident_r[64:128, 64:128]),
                              (vT2[:, sl], ident_r[0:64, 0:64])]
                        for h, (src, idr) in enumerate(vh):
                            vp = pg.tile([128, D], F32R, tag="tr", bufs=2,
                                         name=f"vp{st}_{h}")
                            nc.tensor.transpose(vp[:], src, idr)
                            copy_alt((st + h) % 2 == 0,
                                     v_sb[st][:, h * VW:h * VW + D], vp[:])
                        for h in range(HPC):
                            nc.vector.tensor_copy(
                                v_sb[st][:, h * VW + D:h * VW + D + 2],
                                onecol[:])

            # ---- Phase D: attention + interleaved out-projection ----
            with tc.tile_pool(name="attn_sm", bufs=3) as asm:
                for qb in range(QB):
                    qs = slice(qb * 512, (qb + 1) * 512)
                    kc_max = (qb + 1) * 4
                    for h in range(HPC):
                        if h < 2:
                            kT_h = kT01[h * 64:(h + 1) * 64, :]
                            qT_h = qT01[h * 64:(h + 1) * 64, :]
                        else:
                            kT_h = kT2[:]
                            qT_h = qT2[:]
                        pv = pg.tile([D + 1, 512], F32, tag="pv", bufs=2)
                        for kc in range(kc_max):
                            sp = pg.tile([128, 512], F32, tag="mm512", bufs=4)
                            nc.tensor.matmul(
                                sp[:], kT_h[:, kc * 128:(kc + 1) * 128],
                                qT_h[:, qs])
                            et = ep.tile([128, 512], F32R, tag="est")
                            nc.scalar.activation(et[:], sp[:], AF.Exp,
                                                 scale=SCALE)
                            if kc >= qb * 4:
                                # diagonal tile: zero where k > q
                                nc.gpsimd.affine_select(
                                    out=et[:], in_=et[:],
                                    compare_op=mybir.AluOpType.is_ge,
                                    fill=0.0,
                                    base=qb * 512 - kc * 128,
                                    pattern=[[1, 512]],
                                    channel_multiplier=-1)
                            nc.tensor.matmul(
                                pv[:], v_sb[kc][:, h * VW:h * VW + D + 1],
                                et[:],
                                start=(kc == 0), stop=(kc == kc_max - 1))
                        # stash unnormalized out + normalize via fast recip
                        nc.vector.tensor_copy(oU[h][:, qs], pv[0:64, :])
                        rt = asm.tile([D + 1, 512], F32, tag="rt")
                        nc.vector.tensor_copy(rt[64:65, :], pv[64:65, :])
                        r1 = asm.tile([1, 512], F32, tag="r1")
                        nc.sync.dma_start(r1[:], rt[64:65, :])
                        r1i = asm.tile([1, 512], F32, tag="r1i")
                        nc.vector.reciprocal_approx_fast(r1i[:], r1[:])
                        rb = asm.tile([64, 512], F32, tag="rb")
                        nc.gpsimd.partition_broadcast(rb[:], r1i[:])
                        nc.vector.tensor_mul(oT[h][:, qs], oU[h][:, qs], rb[:])

                    # ---- Phase E (interleaved): out-projection for this qb
                    for st in range(qb * 4, qb * 4 + 4):
                        cs = slice(st * 128, (st + 1) * 128)
                        ob = asm.tile([128, DM], F32, tag="ob", bufs=3)
                        for nh in range(2):
                            ns = slice(nh * 384, (nh + 1) * 384)
                            ps = pg.tile([128, 384], F32, tag="mm512", bufs=4)
                            for h in range(HPC):
                                nc.tensor.matmul(
                                    ps[:], oT[h][:, cs], wo_sb[h][:, ns],
                                    start=(h == 0), stop=(h == HPC - 1))
                            copy_alt(nh == 0, ob[:, ns], ps[:])
                        nc.sync.dma_start(out_d.ap()[cs, :], ob[:])

    nc.compile()
    return nc


def _get_program():
    global _PROG
    if _PROG is None:
        _PROG = _build_program()
    return _PROG


def _shard_inputs(x, Wq, bq, Wk, bk, Wv, bv, Wo):
    in_maps = []
    for core in range(N_CORES):
        b, hg = divmod(core, N_CORES // B)
        fs = slice(hg * HS, (hg + 1) * HS)
        in_maps.append({
            "x": np.ascontiguousarray(x[b], dtype=np.float32),
            "Wq": np.ascontiguousarray(Wq[:, fs], dtype=np.float32),
            "Wk": np.ascontiguousarray(Wk[:, fs], dtype=np.float32),
            "Wv": np.ascontiguousarray(Wv[:, fs], dtype=np.float32),
            "bq": np.ascontiguousarray(bq[fs].reshape(HS, 1), dtype=np.float32),
            "bk": np.ascontiguousarray(bk[fs].reshape(HS, 1), dtype=np.float32),
            "bv": np.ascontiguousarray(bv[fs].reshape(HS, 1), dtype=np.float32),
            "Wo": np.ascontiguousarray(Wo[fs, :], dtype=np.float32),
        })
    return in_maps


def kernel(x, Wq, bq, Wk, bk, Wv, bv, Wo, bo):
    x = np.asarray(x, dtype=np.float32)
    bo = np.asarray(bo, dtype=np.float32)
    in_maps = _shard_inputs(
        x, np.asarray(Wq), np.asarray(bq), np.asarray(Wk), np.asarray(bk),
        np.asarray(Wv), np.asarray(bv), np.asarray(Wo))

    nc = _get_program()
    res = bass_utils.run_bass_kernel_spmd(nc, in_maps,
                                          core_ids=list(range(N_CORES)))

    out = np.zeros((B, S, DM), dtype=np.float32)
    ncg = N_CORES // B
    for b in range(B):
        acc = np.zeros((S, DM), dtype=np.float32)
        for hg in range(ncg):
            acc += res.results[b * ncg + hg]["out"]
        out[b] = acc + bo
    return out
